# revision 21
# baseline (speedup 1.0000x reference)
"""Causal attention (B=4, S=2048, D=1024) on 8 trn2 NeuronCores.

Sharding: core c = (batch b = c//2, query-group h = c%2). Each core handles
one batch and 8 of the 16 query tiles of 128 rows. Tiles are interleaved
(t % 4 in {0,3} for h=0, {1,2} for h=1) so both cores of a pair have the
same causal work profile -> the SPMD program is structurally identical on
every core; per-core differences are data only (query columns + masks).

Math (all matmul inputs bf16, PSUM accum fp32):
  M  = Wq^T Wk / sqrt(D)                      (host, fp32 -> bf16)
  T  = x_q M                                  (device, own queries)
  S  = T x^T  + causal mask                   (keys = raw x, no K proj!)
  P  = exp(S)            (no max subtraction; |logits| <= ~8, fp32 safe)
  rowsum via activation accum_out
  C  = (P x) Wv^T / rowsum                    (associativity: no V proj,
                                               Wv applied on own queries)
"""

import os
import sys

import numpy as np

sys.path.insert(0, "/opt/trn_rl_repo")

import ml_dtypes

import concourse.bass as bass
import concourse.tile as tile
from concourse import bacc, mybir
from concourse.bass_utils import run_bass_kernel_spmd

F32 = mybir.dt.float32
BF16 = mybir.dt.bfloat16
P = 128
B, S, D = 4, 2048, 1024
NDC = D // P                     # 8 contraction chunks of 128
NQT = 8                          # q-tiles of 128 rows per core
QCORE = NQT * P                  # 1024 q rows per core
TILES = {
    0: [t for t in range(16) if t % 4 in (0, 3)],
    1: [t for t in range(16) if t % 4 in (1, 2)],
}
SUPS = [1, 1, 2, 2, 3, 3, 4, 4]  # k-supers (512 wide) per sorted q-tile

_COMPILED = {}
LAST_RESULTS = None


def _emit(nc, tc, ctx, aps):
    x_, xTq, mT, wvT, masks, out = aps
    Exp = mybir.ActivationFunctionType.Exp
    Copy = mybir.ActivationFunctionType.Copy

    copy_ctr = [0]

    def copy_out(dst, src):
        # alternate PSUM->SBUF copies between vector and scalar engines
        # (gpsimd/Pool cannot access PSUM)
        copy_ctr[0] += 1
        if copy_ctr[0] % 2:
            nc.vector.tensor_copy(dst, src)
        else:
            nc.scalar.copy(dst, src)

    pspool = ctx.enter_context(
        tc.tile_pool(name="ps", bufs=2, space=bass.MemorySpace.PSUM)
    )
    cpool = ctx.enter_context(tc.tile_pool(name="const", bufs=1))
    res = ctx.enter_context(tc.tile_pool(name="res", bufs=1))
    psbp = ctx.enter_context(tc.tile_pool(name="psbp", bufs=2))
    pxp = ctx.enter_context(tc.tile_pool(name="pxp", bufs=2))
    pxtp = ctx.enter_context(tc.tile_pool(name="pxtp", bufs=2))
    atp = ctx.enter_context(tc.tile_pool(name="atp", bufs=4))
    csp = ctx.enter_context(tc.tile_pool(name="csp", bufs=2))
    stp = ctx.enter_context(tc.tile_pool(name="stats", bufs=16))

    masksb = cpool.tile([P, NQT, 512], F32)
    xt_sb = res.tile([P, NDC, S], BF16)      # x^T: [d%128, d//128, k]
    x_sb = res.tile([P, S // P, D], BF16)    # x:   [s%128, s//128, d]
    m_sb = res.tile([P, NDC, D], BF16)       # M:   [i%128, i//128, j]
    xtq_sb = res.tile([P, NDC, QCORE], BF16)
    wv_sb = res.tile([P, NDC, D], BF16)      # Wv^T: [d%128, d//128, o]
    tt_sb = res.tile([P, NDC, QCORE], BF16)  # T^T: [j%128, j//128, q]

    # ---- DMA schedule -----------------------------------------------
    # Few, large, multi-dim DMAs: each dma_start costs ~0.6us of sync-
    # sequencer issue time, so 98 small ones serialize into ~60us.
    # Ordered so the T projection can start earliest: M column-slices
    # first (0.25MB each), then query activations.
    def rearr(src):
        return src.rearrange("c p f -> p c f")

    # Single-engine (sync) issue: serialized order doubles as a bandwidth
    # priority list. T-critical data (M slices + xTq) first and alone,
    # then the attention-phase data interleaved by time-of-first-use.
    # x^T is derived from x_ by an XBAR transpose DMA (no host transpose,
    # no separate input).
    # scalar DGE (idle until the first exp ~45us) carries the plain x
    # loads. XBAR transpose DMAs must stay on sync (they wedge the device
    # on the Activation DGE); the big x^T transpose goes last on sync so
    # it does not head-of-line block the small input DMAs.
    nc.scalar.dma_start(x_sb[:, 0:4, :], x_[0:4, :, :].rearrange("s p d -> p s d"))
    nc.scalar.dma_start(x_sb[:, 4:8, :], x_[4:8, :, :].rearrange("s p d -> p s d"))
    nc.scalar.dma_start(x_sb[:, 8:, :], x_[8:, :, :].rearrange("s p d -> p s d"))

    nc.sync.dma_start(m_sb[:, :, 0:P], rearr(mT[:, :, 0:P]))
    nc.sync.dma_start(xtq_sb[:, :, 0:512], rearr(xTq[:, :, 0:512]))
    for co in range(1, NDC):
        nc.sync.dma_start(
            m_sb[:, :, co * P : (co + 1) * P], rearr(mT[:, :, co * P : (co + 1) * P])
        )
    nc.sync.dma_start(xtq_sb[:, :, 512:1024], rearr(xTq[:, :, 512:1024]))
    nc.sync.dma_start(masksb[:], masks[:])
    nc.sync.dma_start(wv_sb[:, :, 0:512], rearr(wvT[:, :, 0:512]))
    nc.sync.dma_start(wv_sb[:, :, 512:1024], rearr(wvT[:, :, 512:1024]))
    nc.sync.dma_start_transpose(xt_sb[:], x_[:].rearrange("s p d -> (s p) d"))

    # ---- T = x_q M  (T^T chunks: [j, q]) ----------------------------
    # qs=0 pass first, co-major: consumes the M column-slice stream.
    for qs in range(2):
        for co in range(NDC):
            ps = pspool.tile([P, 512], F32, tag="mm", bufs=3)
            for ci in range(NDC):
                nc.tensor.matmul(
                    ps[:],
                    m_sb[:, ci, co * P : (co + 1) * P],
                    xtq_sb[:, ci, qs * 512 : (qs + 1) * 512],
                    start=(ci == 0),
                    stop=(ci == NDC - 1),
                )
            # vector-only: scalar's sequencer is occupied by the x loads
            nc.vector.tensor_copy(tt_sb[:, co, qs * 512 : (qs + 1) * 512], ps[:])

    # ---- attention, software-pipelined per q-tile -------------------
    state = {}

    def emit_scores(i):
        n_sup = SUPS[i]
        L = 512 * n_sup
        psb_t = psbp.tile([P, L], BF16, tag="psb", name=f"psb{i}")
        rs_parts = []
        for sup in range(n_sup):
            ps = pspool.tile([P, 512], F32, tag="mm", bufs=3)
            for c in range(NDC):
                nc.tensor.matmul(
                    ps[:],
                    tt_sb[:, c, i * P : (i + 1) * P],
                    xt_sb[:, c, sup * 512 : (sup + 1) * 512],
                    start=(c == 0),
                    stop=(c == NDC - 1),
                )
            if sup == n_sup - 1:
                nc.vector.tensor_add(ps[:], ps[:], masksb[:, i, :])
            rs = stp.tile([P, 1], F32, tag="rs", name=f"rs{i}_{sup}")
            nc.scalar.activation(
                psb_t[:, sup * 512 : (sup + 1) * 512],
                ps[:],
                Exp,
                accum_out=rs[:],
            )
            rs_parts.append(rs)
        acc = rs_parts[0]
        for j, r in enumerate(rs_parts[1:]):
            nxt = stp.tile([P, 1], F32, tag="rs", name=f"rsa{i}_{j}")
            nc.vector.tensor_add(nxt[:], acc[:], r[:])
            acc = nxt
        rcp = stp.tile([P, 1], F32, tag="rcp", name=f"rcp{i}")
        nc.vector.reciprocal(rcp[:], acc[:])
        # P^T chunks via XBAR transpose DMA (replaces PE transposes+copies)
        nkt = L // P
        at_t = atp.tile([P, nkt, P], BF16, tag="at", name=f"at{i}")
        nc.sync.dma_start_transpose(at_t[:], psb_t[:])
        state[i] = (at_t, rcp)

    pxstate = {}

    def emit_px(i):
        # PX = P @ x  -> [q, d] PSUM, stationary P^T chunks from at_t
        n_sup = SUPS[i]
        nkt = (512 * n_sup) // P
        at_t, rcp = state.pop(i)
        px0 = pspool.tile([P, 512], F32, tag="px", bufs=2, name=f"px0_{i}")
        px1 = pspool.tile([P, 512], F32, tag="px", bufs=2, name=f"px1_{i}")
        for k in range(nkt):
            nc.tensor.matmul(
                px0[:], at_t[:, k, :], x_sb[:, k, 0:512],
                start=(k == 0), stop=(k == nkt - 1),
            )
            nc.tensor.matmul(
                px1[:], at_t[:, k, :], x_sb[:, k, 512:1024],
                start=(k == 0), stop=(k == nkt - 1),
            )
        pxsb = pxp.tile([P, D], BF16, tag="pxsb", name=f"pxsb{i}")
        copy_out(pxsb[:, 0:512], px0[:])
        copy_out(pxsb[:, 512:1024], px1[:])
        # PX^T chunks via XBAR transpose DMA
        pxt = pxtp.tile([P, NDC, P], BF16, tag="pxt", name=f"pxt{i}")
        nc.sync.dma_start_transpose(pxt[:], pxsb[:])
        pxstate[i] = (pxt, rcp)

    def emit_ytail(i):
        # C = PX @ Wv^T, scale, store
        pxt, rcp = pxstate.pop(i)
        co0 = pspool.tile([P, 512], F32, tag="co", bufs=3, name=f"co0_{i}")
        co1 = pspool.tile([P, 512], F32, tag="co", bufs=3, name=f"co1_{i}")
        for dc in range(NDC):
            nc.tensor.matmul(
                co0[:], pxt[:, dc, :], wv_sb[:, dc, 0:512],
                start=(dc == 0), stop=(dc == NDC - 1),
            )
            nc.tensor.matmul(
                co1[:], pxt[:, dc, :], wv_sb[:, dc, 512:1024],
                start=(dc == 0), stop=(dc == NDC - 1),
            )
        csb = csp.tile([P, D], BF16, tag="csb", name=f"csb{i}")
        nc.vector.tensor_scalar_mul(csb[:, 0:512], co0[:], rcp[:])
        nc.scalar.activation(csb[:, 512:1024], co1[:], Copy, scale=rcp[:])
        nc.sync.dma_start(out[i * P : (i + 1) * P, :], csb[:])

    # software pipeline: scores(i+2) sits between px(i) / ytail(i) so the
    # PE never waits on the pxsb copy chain
    emit_scores(0)
    emit_scores(1)
    emit_px(0)
    for i in range(2, NQT):
        emit_scores(i)
        emit_ytail(i - 2)
        emit_px(i - 1)
    emit_ytail(NQT - 2)
    emit_px(NQT - 1)
    emit_ytail(NQT - 1)


def _build():
    nc = bacc.Bacc("TRN2", target_bir_lowering=False, debug=False, num_devices=8)

    x_ = nc.dram_tensor("x_", [S // P, P, D], BF16, kind="ExternalInput").ap()
    xTq = nc.dram_tensor("xTq", [NDC, P, QCORE], BF16, kind="ExternalInput").ap()
    mT = nc.dram_tensor("mT", [NDC, P, D], BF16, kind="ExternalInput").ap()
    wvT = nc.dram_tensor("wvT", [NDC, P, D], BF16, kind="ExternalInput").ap()
    masks = nc.dram_tensor("masks", [P, NQT, 512], F32, kind="ExternalInput").ap()
    out = nc.dram_tensor("out", [QCORE, D], BF16, kind="ExternalOutput").ap()

    from contextlib import ExitStack

    with tile.TileContext(nc) as tc, ExitStack() as ctx:
        _emit(nc, tc, ctx, (x_, xTq, mT, wvT, masks, out))

    nc.compile()
    return nc


def _prep_inputs(x, Wk, Wq, Wv):
    bf16 = ml_dtypes.bfloat16
    x = np.asarray(x, np.float32)
    Wk = np.asarray(Wk, np.float32)
    Wq = np.asarray(Wq, np.float32)
    Wv = np.asarray(Wv, np.float32)

    mT = np.ascontiguousarray((Wq.T @ Wk) / (D ** 0.5)).astype(bf16)
    mT = mT.reshape(NDC, P, D)
    wvT = np.ascontiguousarray(Wv.T).astype(bf16).reshape(NDC, P, D)

    mask_by_h = {}
    for h in (0, 1):
        mk = np.empty((P, NQT, 512), np.float32)
        for i, t in enumerate(TILES[h]):
            base = 512 * (SUPS[i] - 1)
            col = base + np.arange(512)[None, :]
            row = t * P + np.arange(P)[:, None]
            mk[:, i, :] = np.where(col <= row, 0.0, -1e30)
        mask_by_h[h] = mk

    in_maps = []
    for c in range(8):
        b, h = c // 2, c % 2
        xb16 = x[b].astype(bf16)
        xTb16 = np.ascontiguousarray(xb16.T)
        qcols = np.concatenate([np.arange(t * P, (t + 1) * P) for t in TILES[h]])
        in_maps.append(
            {
                "x_": xb16.reshape(S // P, P, D),
                "xTq": np.ascontiguousarray(xTb16[:, qcols]).reshape(NDC, P, QCORE),
                "mT": mT,
                "wvT": wvT,
                "masks": mask_by_h[h],
            }
        )
    return in_maps


def kernel(x, Wk, Wq, Wv):
    global LAST_RESULTS
    if 1 not in _COMPILED:
        _COMPILED[1] = _build()
    nc = _COMPILED[1]
    in_maps = _prep_inputs(x, Wk, Wq, Wv)
    trace = bool(int(os.environ.get("BASS_KERNEL_TRACE", "0")))
    res = run_bass_kernel_spmd(nc, in_maps, list(range(8)), trace=trace)
    LAST_RESULTS = res
    out = np.empty((B, S, D), np.float32)
    for c in range(8):
        b, h = c // 2, c % 2
        oc = np.asarray(res.results[c]["out"], np.float32)
        for i, t in enumerate(TILES[h]):
            out[b, t * P : (t + 1) * P, :] = oc[i * P : (i + 1) * P, :]
    return out
